# revision 6
# baseline (speedup 1.0000x reference)
"""Multi-head attention (B=2, S=2048, D=1024, H=16) on 8 trn2 NeuronCores.

Sharding: core c handles batch b = c//4 and head-group g = c%4 (4 heads).
Megatron-style: Wq/Wk/Wv column-split, Wo row-split; host sums the 4 partial
outputs per batch and adds bo.

Per-core math (all matmuls in float32r, fp32 accumulate):
  phase 1: qT/kT = (W^T x^T) in [256, S] layout, v = x W in [S, 260] layout
           (v augmented with a ones column per head -> softmax row sums)
  phase 2: per head pair: S^T[j,i] = kT^T qT (row-packed, 2 heads share the
           PE array), E = exp(S^T), A^T[65, i] += v_aug[j]^T E[j]
           row 64 of A^T = softmax denominators; normalize via reciprocal +
           gpsimd partition_broadcast + DVE multiply
  phase 3: out[i, m] = attn_stack^T Wo_g  (K=256 contraction over 2 tiles)

The 1/sqrt(64) score scale is folded into Wq/bq on the host.
"""

import numpy as np
from contextlib import ExitStack

import concourse.bass as bass
import concourse.tile as tile
from concourse import bacc, mybir
from concourse.bass_utils import run_bass_kernel_spmd

F32 = mybir.dt.float32
F32R = mybir.dt.float32r
AF = mybir.ActivationFunctionType

D_MODEL = 1024
NUM_HEADS = 16
DK = 64
B = 2
S = 2048
NG = 4  # head groups = cores per batch
HPG = 4  # heads per group
CG = HPG * DK  # 256 channels per group
KIN = D_MODEL + 1  # v input channels + ones row
VW = HPG * (DK + 1)  # 260: per-head [v_h | ones]
IB = 512  # i-block (query) width
NI = S // IB
NJ = S // 128
NK = D_MODEL // 128

_CACHE = {}


def build_program():
    nc = bacc.Bacc("TRN2", target_bir_lowering=False, debug=False, num_devices=8)
    xq_d = nc.dram_tensor("xq", [D_MODEL, S], F32R, kind="ExternalInput")
    xk_d = nc.dram_tensor("xk", [D_MODEL, S], F32R, kind="ExternalInput")
    xv_d = nc.dram_tensor("xv", [KIN, S], F32R, kind="ExternalInput")
    wq_d = nc.dram_tensor("wq", [D_MODEL, CG], F32R, kind="ExternalInput")
    wk_d = nc.dram_tensor("wk", [D_MODEL, CG], F32R, kind="ExternalInput")
    bqk_d = nc.dram_tensor("bqk", [CG, 2], F32, kind="ExternalInput")
    wv_d = nc.dram_tensor("wv", [KIN, VW], F32R, kind="ExternalInput")
    wo_d = nc.dram_tensor("wo", [CG, D_MODEL], F32R, kind="ExternalInput")
    out_d = nc.dram_tensor("out", [S, D_MODEL], F32, kind="ExternalOutput")

    with tile.TileContext(nc) as tc, ExitStack() as ctx:
        wpool = ctx.enter_context(tc.tile_pool(name="wpool", bufs=1))
        qkvpool = ctx.enter_context(tc.tile_pool(name="qkv", bufs=1))
        attnpool = ctx.enter_context(tc.tile_pool(name="attn", bufs=1))

        # ---- resident weights ----
        wq_sb = wpool.tile([128, NK * CG], F32R)  # k-tile k at cols [CG*k, CG*(k+1))
        wk_sb = wpool.tile([128, NK * CG], F32R)
        for k in range(NK):
            nc.sync.dma_start(
                wq_sb[:, k * CG : (k + 1) * CG], wq_d.ap()[k * 128 : (k + 1) * 128, :]
            )
            nc.sync.dma_start(
                wk_sb[:, k * CG : (k + 1) * CG], wk_d.ap()[k * 128 : (k + 1) * 128, :]
            )
        wv_sb = wpool.tile([128, NK * VW], F32R)
        for k in range(NK):
            nc.sync.dma_start(
                wv_sb[:, k * VW : (k + 1) * VW], wv_d.ap()[k * 128 : (k + 1) * 128, :]
            )
        wvb = wpool.tile([1, VW], F32R)  # v bias row (incl. ones-col entries)
        nc.sync.dma_start(wvb[:], wv_d.ap()[KIN - 1 : KIN, :])
        bq_sb = {}
        bk_sb = {}
        for cb in range(2):
            bq_sb[cb] = wpool.tile([128, 1], F32, name=f"bq{cb}")
            nc.sync.dma_start(bq_sb[cb][:], bqk_d.ap()[cb * 128 : (cb + 1) * 128, 0:1])
            bk_sb[cb] = wpool.tile([128, 1], F32, name=f"bk{cb}")
            nc.sync.dma_start(bk_sb[cb][:], bqk_d.ap()[cb * 128 : (cb + 1) * 128, 1:2])

        # ---- phase 1 outputs (resident) ----
        qT = [qkvpool.tile([128, S], F32R, name=f"qT{t}") for t in range(2)]
        kT = [qkvpool.tile([128, S], F32R, name=f"kT{t}") for t in range(2)]
        v_sb = qkvpool.tile([128, NJ * VW], F32R)
        att = [attnpool.tile([128, S], F32R, name=f"att{t}") for t in range(2)]

        with (
            tc.tile_pool(name="ph1psum", bufs=2, space="PSUM") as ph1psum,
            tc.tile_pool(name="xvpool", bufs=1) as xvpool,
            tc.tile_pool(name="xspool", bufs=8) as xspool,
        ):
            # ---- v projection: v[j, e] = sum_k xv[k, j] wv[k, e] ----
            xv_sb = [xvpool.tile([128, S], F32R, name=f"xv{k}") for k in range(NK)]
            for k in range(NK):
                nc.sync.dma_start(xv_sb[k][:], xv_d.ap()[k * 128 : (k + 1) * 128, :])
            xvon = xvpool.tile([1, S], F32R)  # ones row
            nc.sync.dma_start(xvon[:], xv_d.ap()[KIN - 1 : KIN, :])

            for j in range(NJ):
                pv = ph1psum.tile([128, VW], F32, name="p0")
                for k in range(NK):
                    nc.tensor.matmul(
                        pv[:],
                        xv_sb[k][:, j * 128 : (j + 1) * 128],
                        wv_sb[:, k * VW : (k + 1) * VW],
                        start=(k == 0),
                        stop=False,
                    )
                nc.tensor.matmul(
                    pv[:],
                    xvon[0:1, j * 128 : (j + 1) * 128],
                    wvb[:],
                    start=False,
                    stop=True,
                )
                nc.vector.tensor_copy(v_sb[:, j * VW : (j + 1) * VW], pv[:])

            # ---- q/k projections: qT[c, i] = sum_k wq[k, c] x[k, i] (+bias) ----
            for xdram, w_sb, b_sb, dst in (
                (xq_d, wq_sb, bq_sb, qT),
                (xk_d, wk_sb, bk_sb, kT),
            ):
                for i in range(NI):
                    xt = {}
                    for k in range(NK):
                        xt[k] = xspool.tile([128, IB], F32R, name="xs")
                        nc.sync.dma_start(
                            xt[k][:],
                            xdram.ap()[k * 128 : (k + 1) * 128, i * IB : (i + 1) * IB],
                        )
                    for cb in range(2):
                        pq = ph1psum.tile([128, IB], F32, name=f"p{cb}")
                        for k in range(NK):
                            nc.tensor.matmul(
                                pq[:],
                                w_sb[:, k * CG + cb * 128 : k * CG + (cb + 1) * 128],
                                xt[k][:],
                                start=(k == 0),
                                stop=(k == NK - 1),
                            )
                        nc.scalar.activation(
                            dst[cb][:, i * IB : (i + 1) * IB],
                            pq[:],
                            AF.Identity,
                            bias=b_sb[cb][:],
                        )

        # wo loaded after phase-1 pools release; DMA overlaps phase 2
        wopool = ctx.enter_context(tc.tile_pool(name="wopool", bufs=1))
        wo_sb = [wopool.tile([128, D_MODEL], F32R, name=f"wo{t}") for t in range(2)]
        for t in range(2):
            nc.sync.dma_start(wo_sb[t][:], wo_d.ap()[t * 128 : (t + 1) * 128, :])

        # ---- phase 2: attention per head pair ----
        with (
            tc.tile_pool(name="spsum", bufs=2, space="PSUM") as spsum,
            tc.tile_pool(name="apsum", bufs=2, space="PSUM") as apsum,
            tc.tile_pool(name="epool", bufs=3) as epool,
            tc.tile_pool(name="rpool", bufs=2) as rpool,
        ):
            for pr in range(2):  # heads (2pr, 2pr+1), channels in qT[pr]/kT[pr]
                for i in range(NI):
                    aa = {}
                    for u in range(2):
                        aa[u] = apsum.tile([DK + 1, IB], F32, name=f"a{u}")
                    for j in range(NJ):
                        ss = {}
                        for u in range(2):
                            ss[u] = spsum.tile([128, IB], F32, name=f"s{u}")
                            nc.tensor.matmul(
                                ss[u][:],
                                kT[pr][u * DK : (u + 1) * DK, j * 128 : (j + 1) * 128],
                                qT[pr][u * DK : (u + 1) * DK, i * IB : (i + 1) * IB],
                                start=True,
                                stop=True,
                                tile_position=(u * DK, 0),
                            )
                        for u in range(2):
                            ee = epool.tile([128, IB], F32R, name=f"e{u}")
                            nc.scalar.activation(ee[:], ss[u][:], AF.Exp)
                            h = 2 * pr + u  # local head
                            nc.tensor.matmul(
                                aa[u][:],
                                v_sb[:, j * VW + h * (DK + 1) : j * VW + (h + 1) * (DK + 1)],
                                ee[:],
                                start=(j == 0),
                                stop=(j == NJ - 1),
                            )
                    for u in range(2):
                        h = 2 * pr + u
                        # reciprocal of the sums row (PSUM partition 64), kept
                        # at base 64 so the DVE needs no partition shift; the
                        # row then moves to partition 0 by DMA because gpsimd
                        # partition_broadcast only accepts a base-0 source
                        r = rpool.tile([128, IB], F32, name="r")
                        nc.vector.reciprocal(r[DK : DK + 1, :], aa[u][DK : DK + 1, :])
                        r0 = rpool.tile([1, IB], F32, name="r0")
                        nc.sync.dma_start(r0[:], r[DK : DK + 1, :])
                        rb = rpool.tile([DK, IB], F32, name="rb")
                        nc.gpsimd.partition_broadcast(rb[:], r0[:])
                        t, po = h // 2, 64 * (h % 2)
                        if po == 0:
                            nc.vector.tensor_mul(
                                att[t][0:DK, i * IB : (i + 1) * IB], aa[u][0:DK, :], rb[:]
                            )
                        else:
                            nrm = rpool.tile([DK, IB], F32R, name="nrm")
                            nc.vector.tensor_mul(nrm[:], aa[u][0:DK, :], rb[:])
                            nc.sync.dma_start(
                                att[t][po : po + DK, i * IB : (i + 1) * IB], nrm[:]
                            )

        # ---- phase 3: out[i, m] = sum_c att[c, i] wo[c, m] ----
        with (
            tc.tile_pool(name="opsum", bufs=2, space="PSUM") as opsum,
            tc.tile_pool(name="obounce", bufs=4) as obounce,
        ):
            for ib in range(S // 128):
                for mh in range(2):
                    po = opsum.tile([128, IB], F32, name=f"o{mh}")
                    for t in range(2):
                        nc.tensor.matmul(
                            po[:],
                            att[t][:, ib * 128 : (ib + 1) * 128],
                            wo_sb[t][:, mh * IB : (mh + 1) * IB],
                            start=(t == 0),
                            stop=(t == 1),
                        )
                    ob = obounce.tile([128, IB], F32, name="ob")
                    nc.vector.tensor_copy(ob[:], po[:])
                    nc.sync.dma_start(
                        out_d.ap()[ib * 128 : (ib + 1) * 128, mh * IB : (mh + 1) * IB],
                        ob[:],
                    )

    nc.compile()
    return nc


def _prep_inputs(Q, K, V, Wq, bq, Wk, bk, Wv, bv, Wo, bo):
    """Build the 8 per-core input maps (host-side shard + layout)."""
    ones = np.ones((1, S), dtype=np.float32)
    per_batch = []
    for b in range(B):
        xq = np.ascontiguousarray(Q[b].T)
        xk = np.ascontiguousarray(K[b].T)
        xv = np.concatenate([np.ascontiguousarray(V[b].T), ones], axis=0)
        per_batch.append((xq, xk, xv))
    in_maps = []
    for c in range(8):
        b, g = divmod(c, NG)
        xq, xk, xv = per_batch[b]
        gs = slice(g * CG, (g + 1) * CG)
        wq = np.ascontiguousarray(Wq[:, gs]) * 0.125
        wk = np.ascontiguousarray(Wk[:, gs])
        bqk = np.stack([bq[gs] * 0.125, bk[gs]], axis=1).astype(np.float32)
        wv = np.zeros((KIN, VW), dtype=np.float32)
        for e in range(HPG):
            wv[:D_MODEL, e * (DK + 1) : e * (DK + 1) + DK] = Wv[
                :, g * CG + e * DK : g * CG + (e + 1) * DK
            ]
            wv[D_MODEL, e * (DK + 1) : e * (DK + 1) + DK] = bv[
                g * CG + e * DK : g * CG + (e + 1) * DK
            ]
            wv[D_MODEL, e * (DK + 1) + DK] = 1.0
        wo = np.ascontiguousarray(Wo[g * CG : (g + 1) * CG, :])
        in_maps.append(
            {
                "xq": xq,
                "xk": xk,
                "xv": np.ascontiguousarray(xv),
                "wq": wq.astype(np.float32),
                "wk": wk.astype(np.float32),
                "bqk": bqk,
                "wv": wv,
                "wo": wo.astype(np.float32),
            }
        )
    return in_maps


def run(inputs, trace=False):
    if "nc" not in _CACHE:
        _CACHE["nc"] = build_program()
    nc = _CACHE["nc"]
    in_maps = _prep_inputs(**inputs)
    res = run_bass_kernel_spmd(nc, in_maps, core_ids=list(range(8)), trace=trace)
    bo = np.asarray(inputs["bo"], dtype=np.float32)
    outs = []
    for b in range(B):
        acc = res.results[4 * b]["out"].astype(np.float32)
        for g in range(1, NG):
            acc = acc + res.results[4 * b + g]["out"]
        outs.append(acc + bo[None, :])
    return np.stack(outs, axis=0), res


def kernel(**inputs):
    inputs = {k: np.asarray(v) for k, v in inputs.items()}
    out, _ = run(inputs, trace=False)
    return out.astype(np.float32)


# revision 8
# speedup vs baseline: 1.0088x; 1.0088x over previous
"""Multi-head attention (B=2, S=2048, D=1024, H=16) on 8 trn2 NeuronCores.

Sharding: core c handles batch b = c//4 and head-group g = c%4 (4 heads).
Megatron-style: Wq/Wk/Wv column-split, Wo row-split; host sums the 4 partial
outputs per batch and adds bo.

Per-core math (all matmuls in float32r, fp32 accumulate):
  phase 1: qT/kT = (W^T x^T) in [256, S] layout, v = x W in [S, 260] layout
           (v augmented with a ones column per head -> softmax row sums)
  phase 2: per head pair: S^T[j,i] = kT^T qT (row-packed, 2 heads share the
           PE array), E = exp(S^T), A^T[65, i] += v_aug[j]^T E[j]
           row 64 of A^T = softmax denominators; normalize via reciprocal +
           gpsimd partition_broadcast + DVE multiply
  phase 3: out[i, m] = attn_stack^T Wo_g  (K=256 contraction over 2 tiles)

The 1/sqrt(64) score scale is folded into Wq/bq on the host.
"""

import numpy as np
from contextlib import ExitStack

import concourse.bass as bass
import concourse.tile as tile
from concourse import bacc, mybir
from concourse.bass_utils import run_bass_kernel_spmd

F32 = mybir.dt.float32
F32R = mybir.dt.float32r
AF = mybir.ActivationFunctionType

D_MODEL = 1024
NUM_HEADS = 16
DK = 64
B = 2
S = 2048
NG = 4  # head groups = cores per batch
HPG = 4  # heads per group
CG = HPG * DK  # 256 channels per group
KIN = D_MODEL + 1  # v input channels + ones row
VW = HPG * (DK + 1)  # 260: per-head [v_h | ones]
IB = 512  # i-block (query) width
NI = S // IB
NJ = S // 128
NK = D_MODEL // 128

_CACHE = {}


def build_program():
    nc = bacc.Bacc("TRN2", target_bir_lowering=False, debug=False, num_devices=8)
    xq_d = nc.dram_tensor("xq", [D_MODEL, S], F32R, kind="ExternalInput")
    xk_d = nc.dram_tensor("xk", [D_MODEL, S], F32R, kind="ExternalInput")
    xv_d = nc.dram_tensor("xv", [KIN, S], F32R, kind="ExternalInput")
    wq_d = nc.dram_tensor("wq", [D_MODEL, CG], F32R, kind="ExternalInput")
    wk_d = nc.dram_tensor("wk", [D_MODEL, CG], F32R, kind="ExternalInput")
    bqk_d = nc.dram_tensor("bqk", [CG, 2], F32, kind="ExternalInput")
    wv_d = nc.dram_tensor("wv", [KIN, VW], F32R, kind="ExternalInput")
    wo_d = nc.dram_tensor("wo", [CG, D_MODEL], F32R, kind="ExternalInput")
    out_d = nc.dram_tensor("out", [S, D_MODEL], F32, kind="ExternalOutput")

    with tile.TileContext(nc) as tc, ExitStack() as ctx:
        wpool = ctx.enter_context(tc.tile_pool(name="wpool", bufs=1))
        qkvpool = ctx.enter_context(tc.tile_pool(name="qkv", bufs=1))
        attnpool = ctx.enter_context(tc.tile_pool(name="attn", bufs=1))

        # ---- resident weights ----
        wq_sb = wpool.tile([128, NK * CG], F32R)  # k-tile k at cols [CG*k, CG*(k+1))
        wk_sb = wpool.tile([128, NK * CG], F32R)
        for k in range(NK):
            nc.sync.dma_start(
                wq_sb[:, k * CG : (k + 1) * CG], wq_d.ap()[k * 128 : (k + 1) * 128, :]
            )
            nc.sync.dma_start(
                wk_sb[:, k * CG : (k + 1) * CG], wk_d.ap()[k * 128 : (k + 1) * 128, :]
            )
        wv_sb = wpool.tile([128, NK * VW], F32R)
        for k in range(NK):
            nc.sync.dma_start(
                wv_sb[:, k * VW : (k + 1) * VW], wv_d.ap()[k * 128 : (k + 1) * 128, :]
            )
        wvb = wpool.tile([1, VW], F32R)  # v bias row (incl. ones-col entries)
        nc.sync.dma_start(wvb[:], wv_d.ap()[KIN - 1 : KIN, :])
        bq_sb = {}
        bk_sb = {}
        for cb in range(2):
            bq_sb[cb] = wpool.tile([128, 1], F32, name=f"bq{cb}")
            nc.sync.dma_start(bq_sb[cb][:], bqk_d.ap()[cb * 128 : (cb + 1) * 128, 0:1])
            bk_sb[cb] = wpool.tile([128, 1], F32, name=f"bk{cb}")
            nc.sync.dma_start(bk_sb[cb][:], bqk_d.ap()[cb * 128 : (cb + 1) * 128, 1:2])

        # ---- phase 1 outputs (resident) ----
        qT = [qkvpool.tile([128, S], F32R, name=f"qT{t}") for t in range(2)]
        kT = [qkvpool.tile([128, S], F32R, name=f"kT{t}") for t in range(2)]
        v_sb = qkvpool.tile([128, NJ * VW], F32R)
        att = [attnpool.tile([128, S], F32R, name=f"att{t}") for t in range(2)]

        _sid1, _ = nc.enter_named_scope("phase1", False)
        with (
            tc.tile_pool(name="ph1psum", bufs=2, space="PSUM") as ph1psum,
            tc.tile_pool(name="xvpool", bufs=1) as xvpool,
            tc.tile_pool(name="xspool", bufs=8) as xspool,
        ):
            # ---- v projection: v[j, e] = sum_k xv[k, j] wv[k, e] ----
            xv_sb = [xvpool.tile([128, S], F32R, name=f"xv{k}") for k in range(NK)]
            for k in range(NK):
                nc.sync.dma_start(xv_sb[k][:], xv_d.ap()[k * 128 : (k + 1) * 128, :])
            xvon = xvpool.tile([1, S], F32R)  # ones row
            nc.sync.dma_start(xvon[:], xv_d.ap()[KIN - 1 : KIN, :])

            for j in range(NJ):
                pv = ph1psum.tile([128, VW], F32, name="p0")
                for k in range(NK):
                    nc.tensor.matmul(
                        pv[:],
                        xv_sb[k][:, j * 128 : (j + 1) * 128],
                        wv_sb[:, k * VW : (k + 1) * VW],
                        start=(k == 0),
                        stop=False,
                    )
                nc.tensor.matmul(
                    pv[:],
                    xvon[0:1, j * 128 : (j + 1) * 128],
                    wvb[:],
                    start=False,
                    stop=True,
                )
                nc.vector.tensor_copy(v_sb[:, j * VW : (j + 1) * VW], pv[:])

            # ---- q/k projections: qT[c, i] = sum_k wq[k, c] x[k, i] (+bias) ----
            for xdram, w_sb, b_sb, dst in (
                (xq_d, wq_sb, bq_sb, qT),
                (xk_d, wk_sb, bk_sb, kT),
            ):
                for i in range(NI):
                    xt = {}
                    for k in range(NK):
                        xt[k] = xspool.tile([128, IB], F32R, name="xs")
                        nc.sync.dma_start(
                            xt[k][:],
                            xdram.ap()[k * 128 : (k + 1) * 128, i * IB : (i + 1) * IB],
                        )
                    for cb in range(2):
                        pq = ph1psum.tile([128, IB], F32, name=f"p{cb}")
                        for k in range(NK):
                            nc.tensor.matmul(
                                pq[:],
                                w_sb[:, k * CG + cb * 128 : k * CG + (cb + 1) * 128],
                                xt[k][:],
                                start=(k == 0),
                                stop=(k == NK - 1),
                            )
                        nc.scalar.activation(
                            dst[cb][:, i * IB : (i + 1) * IB],
                            pq[:],
                            AF.Identity,
                            bias=b_sb[cb][:],
                        )

        nc.leave_named_scope("phase1", _sid1, False)

        # wo loaded after phase-1 pools release; DMA overlaps phase 2
        wopool = ctx.enter_context(tc.tile_pool(name="wopool", bufs=1))
        wo_sb = [wopool.tile([128, D_MODEL], F32R, name=f"wo{t}") for t in range(2)]
        for t in range(2):
            nc.sync.dma_start(wo_sb[t][:], wo_d.ap()[t * 128 : (t + 1) * 128, :])

        # ---- phase 2: attention per head pair ----
        _sid2, _ = nc.enter_named_scope("phase2", False)
        with (
            tc.tile_pool(name="spsum", bufs=2, space="PSUM") as spsum,
            tc.tile_pool(name="apsum", bufs=2, space="PSUM") as apsum,
            tc.tile_pool(name="epool", bufs=3) as epool,
            tc.tile_pool(name="rpool", bufs=2) as rpool,
        ):
            for pr in range(2):  # heads (2pr, 2pr+1), channels in qT[pr]/kT[pr]
                for i in range(NI):
                    aa = {}
                    for u in range(2):
                        aa[u] = apsum.tile([DK + 1, IB], F32, name=f"a{u}")
                    for j in range(NJ):
                        ss = {}
                        for u in range(2):
                            ss[u] = spsum.tile([128, IB], F32, name=f"s{u}")
                            nc.tensor.matmul(
                                ss[u][:],
                                kT[pr][u * DK : (u + 1) * DK, j * 128 : (j + 1) * 128],
                                qT[pr][u * DK : (u + 1) * DK, i * IB : (i + 1) * IB],
                                start=True,
                                stop=True,
                                tile_position=(u * DK, 0),
                            )
                        for u in range(2):
                            ee = epool.tile([128, IB], F32R, name=f"e{u}")
                            nc.scalar.activation(ee[:], ss[u][:], AF.Exp)
                            h = 2 * pr + u  # local head
                            nc.tensor.matmul(
                                aa[u][:],
                                v_sb[:, j * VW + h * (DK + 1) : j * VW + (h + 1) * (DK + 1)],
                                ee[:],
                                start=(j == 0),
                                stop=(j == NJ - 1),
                            )
                    for u in range(2):
                        h = 2 * pr + u
                        # reciprocal of the sums row (PSUM partition 64), kept
                        # at base 64 so the DVE needs no partition shift; the
                        # row then moves to partition 0 by DMA because gpsimd
                        # partition_broadcast only accepts a base-0 source
                        r = rpool.tile([128, IB], F32, name="r")
                        nc.vector.reciprocal(r[DK : DK + 1, :], aa[u][DK : DK + 1, :])
                        r0 = rpool.tile([1, IB], F32, name="r0")
                        nc.sync.dma_start(r0[:], r[DK : DK + 1, :])
                        rb = rpool.tile([DK, IB], F32, name="rb")
                        nc.gpsimd.partition_broadcast(rb[:], r0[:])
                        t, po = h // 2, 64 * (h % 2)
                        if po == 0:
                            nc.vector.tensor_mul(
                                att[t][0:DK, i * IB : (i + 1) * IB], aa[u][0:DK, :], rb[:]
                            )
                        else:
                            nrm = rpool.tile([DK, IB], F32R, name="nrm")
                            nc.vector.tensor_mul(nrm[:], aa[u][0:DK, :], rb[:])
                            nc.sync.dma_start(
                                att[t][po : po + DK, i * IB : (i + 1) * IB], nrm[:]
                            )

        nc.leave_named_scope("phase2", _sid2, False)

        # ---- phase 3: out[i, m] = sum_c att[c, i] wo[c, m] ----
        _sid3, _ = nc.enter_named_scope("phase3", False)
        with (
            tc.tile_pool(name="opsum", bufs=2, space="PSUM") as opsum,
            tc.tile_pool(name="obounce", bufs=4) as obounce,
        ):
            for ib in range(S // 128):
                for mh in range(2):
                    po = opsum.tile([128, IB], F32, name=f"o{mh}")
                    for t in range(2):
                        nc.tensor.matmul(
                            po[:],
                            att[t][:, ib * 128 : (ib + 1) * 128],
                            wo_sb[t][:, mh * IB : (mh + 1) * IB],
                            start=(t == 0),
                            stop=(t == 1),
                        )
                    ob = obounce.tile([128, IB], F32, name="ob")
                    nc.vector.tensor_copy(ob[:], po[:])
                    nc.sync.dma_start(
                        out_d.ap()[ib * 128 : (ib + 1) * 128, mh * IB : (mh + 1) * IB],
                        ob[:],
                    )
        nc.leave_named_scope("phase3", _sid3, False)

    nc.compile()
    return nc


def _prep_inputs(Q, K, V, Wq, bq, Wk, bk, Wv, bv, Wo, bo):
    """Build the 8 per-core input maps (host-side shard + layout)."""
    ones = np.ones((1, S), dtype=np.float32)
    per_batch = []
    for b in range(B):
        xq = np.ascontiguousarray(Q[b].T)
        xk = np.ascontiguousarray(K[b].T)
        xv = np.concatenate([np.ascontiguousarray(V[b].T), ones], axis=0)
        per_batch.append((xq, xk, xv))
    in_maps = []
    for c in range(8):
        b, g = divmod(c, NG)
        xq, xk, xv = per_batch[b]
        gs = slice(g * CG, (g + 1) * CG)
        wq = np.ascontiguousarray(Wq[:, gs]) * 0.125
        wk = np.ascontiguousarray(Wk[:, gs])
        bqk = np.stack([bq[gs] * 0.125, bk[gs]], axis=1).astype(np.float32)
        wv = np.zeros((KIN, VW), dtype=np.float32)
        for e in range(HPG):
            wv[:D_MODEL, e * (DK + 1) : e * (DK + 1) + DK] = Wv[
                :, g * CG + e * DK : g * CG + (e + 1) * DK
            ]
            wv[D_MODEL, e * (DK + 1) : e * (DK + 1) + DK] = bv[
                g * CG + e * DK : g * CG + (e + 1) * DK
            ]
            wv[D_MODEL, e * (DK + 1) + DK] = 1.0
        wo = np.ascontiguousarray(Wo[g * CG : (g + 1) * CG, :])
        in_maps.append(
            {
                "xq": xq,
                "xk": xk,
                "xv": np.ascontiguousarray(xv),
                "wq": wq.astype(np.float32),
                "wk": wk.astype(np.float32),
                "bqk": bqk,
                "wv": wv,
                "wo": wo.astype(np.float32),
            }
        )
    return in_maps


def run(inputs, trace=False):
    if "nc" not in _CACHE:
        _CACHE["nc"] = build_program()
    nc = _CACHE["nc"]
    in_maps = _prep_inputs(**inputs)
    res = run_bass_kernel_spmd(nc, in_maps, core_ids=list(range(8)), trace=trace)
    bo = np.asarray(inputs["bo"], dtype=np.float32)
    outs = []
    for b in range(B):
        acc = res.results[4 * b]["out"].astype(np.float32)
        for g in range(1, NG):
            acc = acc + res.results[4 * b + g]["out"]
        outs.append(acc + bo[None, :])
    return np.stack(outs, axis=0), res


def kernel(**inputs):
    inputs = {k: np.asarray(v) for k, v in inputs.items()}
    out, _ = run(inputs, trace=False)
    return out.astype(np.float32)


# revision 9
# speedup vs baseline: 1.7507x; 1.7354x over previous
"""Multi-head attention (B=2, S=2048, D=1024, H=16) on 8 trn2 NeuronCores.

Sharding: core c handles batch b = c//4 and head-group g = c%4 (4 heads).
Megatron-style: Wq/Wk/Wv column-split, Wo row-split; host sums the 4 partial
outputs per batch and adds bo.

Per-core math (all matmuls in float32r, fp32 accumulate):
  phase 1: qT/kT = (W^T x^T) in [256, S] layout, v = x W in [S, 260] layout
           (v augmented with a ones column per head -> softmax row sums)
  phase 2: per head pair: S^T[j,i] = kT^T qT (row-packed, 2 heads share the
           PE array), E = exp(S^T), A^T[65, i] += v_aug[j]^T E[j]
           row 64 of A^T = softmax denominators; normalize via reciprocal +
           gpsimd partition_broadcast + DVE multiply
  phase 3: out[i, m] = attn_stack^T Wo_g  (K=256 contraction over 2 tiles)

The 1/sqrt(64) score scale is folded into Wq/bq on the host.
"""

import numpy as np
import ml_dtypes
from contextlib import ExitStack

import concourse.bass as bass
import concourse.tile as tile
from concourse import bacc, mybir
from concourse.bass_utils import run_bass_kernel_spmd

F32 = mybir.dt.float32
BF16 = mybir.dt.bfloat16
AF = mybir.ActivationFunctionType

D_MODEL = 1024
NUM_HEADS = 16
DK = 64
B = 2
S = 2048
NG = 4  # head groups = cores per batch
HPG = 4  # heads per group
CG = HPG * DK  # 256 channels per group
KIN = D_MODEL + 1  # v input channels + ones row
VW = HPG * (DK + 1)  # 260: per-head [v_h | ones]
IB = 512  # i-block (query) width
NI = S // IB
NJ = S // 128
NK = D_MODEL // 128

_CACHE = {}


def build_program():
    nc = bacc.Bacc("TRN2", target_bir_lowering=False, debug=False, num_devices=8)
    xq_d = nc.dram_tensor("xq", [D_MODEL, S], BF16, kind="ExternalInput")
    xk_d = nc.dram_tensor("xk", [D_MODEL, S], BF16, kind="ExternalInput")
    xv_d = nc.dram_tensor("xv", [KIN, S], BF16, kind="ExternalInput")
    wq_d = nc.dram_tensor("wq", [D_MODEL, CG], BF16, kind="ExternalInput")
    wk_d = nc.dram_tensor("wk", [D_MODEL, CG], BF16, kind="ExternalInput")
    bqk_d = nc.dram_tensor("bqk", [CG, 2], F32, kind="ExternalInput")
    wv_d = nc.dram_tensor("wv", [KIN, VW], BF16, kind="ExternalInput")
    wo_d = nc.dram_tensor("wo", [CG, D_MODEL], BF16, kind="ExternalInput")
    out_d = nc.dram_tensor("out", [S, D_MODEL], F32, kind="ExternalOutput")

    with tile.TileContext(nc) as tc, ExitStack() as ctx:
        wpool = ctx.enter_context(tc.tile_pool(name="wpool", bufs=1))
        qkvpool = ctx.enter_context(tc.tile_pool(name="qkv", bufs=1))
        attnpool = ctx.enter_context(tc.tile_pool(name="attn", bufs=1))

        # ---- resident weights ----
        wq_sb = wpool.tile([128, NK * CG], BF16)  # k-tile k at cols [CG*k, CG*(k+1))
        wk_sb = wpool.tile([128, NK * CG], BF16)
        for k in range(NK):
            nc.sync.dma_start(
                wq_sb[:, k * CG : (k + 1) * CG], wq_d.ap()[k * 128 : (k + 1) * 128, :]
            )
            nc.sync.dma_start(
                wk_sb[:, k * CG : (k + 1) * CG], wk_d.ap()[k * 128 : (k + 1) * 128, :]
            )
        wv_sb = wpool.tile([128, NK * VW], BF16)
        for k in range(NK):
            nc.sync.dma_start(
                wv_sb[:, k * VW : (k + 1) * VW], wv_d.ap()[k * 128 : (k + 1) * 128, :]
            )
        wvb = wpool.tile([1, VW], BF16)  # v bias row (incl. ones-col entries)
        nc.sync.dma_start(wvb[:], wv_d.ap()[KIN - 1 : KIN, :])
        bq_sb = {}
        bk_sb = {}
        for cb in range(2):
            bq_sb[cb] = wpool.tile([128, 1], F32, name=f"bq{cb}")
            nc.sync.dma_start(bq_sb[cb][:], bqk_d.ap()[cb * 128 : (cb + 1) * 128, 0:1])
            bk_sb[cb] = wpool.tile([128, 1], F32, name=f"bk{cb}")
            nc.sync.dma_start(bk_sb[cb][:], bqk_d.ap()[cb * 128 : (cb + 1) * 128, 1:2])

        # ---- phase 1 outputs (resident) ----
        qT = [qkvpool.tile([128, S], BF16, name=f"qT{t}") for t in range(2)]
        kT = [qkvpool.tile([128, S], BF16, name=f"kT{t}") for t in range(2)]
        v_sb = qkvpool.tile([128, NJ * VW], BF16)
        att = [attnpool.tile([128, S], BF16, name=f"att{t}") for t in range(2)]

        _sid1, _ = nc.enter_named_scope("phase1", False)
        with (
            tc.tile_pool(name="ph1psum", bufs=2, space="PSUM") as ph1psum,
            tc.tile_pool(name="xvpool", bufs=1) as xvpool,
            tc.tile_pool(name="xspool", bufs=8) as xspool,
        ):
            # ---- v projection: v[j, e] = sum_k xv[k, j] wv[k, e] ----
            xv_sb = [xvpool.tile([128, S], BF16, name=f"xv{k}") for k in range(NK)]
            for k in range(NK):
                nc.sync.dma_start(xv_sb[k][:], xv_d.ap()[k * 128 : (k + 1) * 128, :])
            xvon = xvpool.tile([1, S], BF16)  # ones row
            nc.sync.dma_start(xvon[:], xv_d.ap()[KIN - 1 : KIN, :])

            for j in range(NJ):
                pv = ph1psum.tile([128, VW], F32, name="p0")
                for k in range(NK):
                    nc.tensor.matmul(
                        pv[:],
                        xv_sb[k][:, j * 128 : (j + 1) * 128],
                        wv_sb[:, k * VW : (k + 1) * VW],
                        start=(k == 0),
                        stop=False,
                    )
                nc.tensor.matmul(
                    pv[:],
                    xvon[0:1, j * 128 : (j + 1) * 128],
                    wvb[:],
                    start=False,
                    stop=True,
                )
                nc.vector.tensor_copy(v_sb[:, j * VW : (j + 1) * VW], pv[:])

            # ---- q/k projections: qT[c, i] = sum_k wq[k, c] x[k, i] (+bias) ----
            for xdram, w_sb, b_sb, dst in (
                (xq_d, wq_sb, bq_sb, qT),
                (xk_d, wk_sb, bk_sb, kT),
            ):
                for i in range(NI):
                    xt = {}
                    for k in range(NK):
                        xt[k] = xspool.tile([128, IB], BF16, name="xs")
                        nc.sync.dma_start(
                            xt[k][:],
                            xdram.ap()[k * 128 : (k + 1) * 128, i * IB : (i + 1) * IB],
                        )
                    for cb in range(2):
                        pq = ph1psum.tile([128, IB], F32, name=f"p{cb}")
                        for k in range(NK):
                            nc.tensor.matmul(
                                pq[:],
                                w_sb[:, k * CG + cb * 128 : k * CG + (cb + 1) * 128],
                                xt[k][:],
                                start=(k == 0),
                                stop=(k == NK - 1),
                            )
                        nc.scalar.activation(
                            dst[cb][:, i * IB : (i + 1) * IB],
                            pq[:],
                            AF.Identity,
                            bias=b_sb[cb][:],
                        )

        nc.leave_named_scope("phase1", _sid1, False)

        # wo loaded after phase-1 pools release; DMA overlaps phase 2
        wopool = ctx.enter_context(tc.tile_pool(name="wopool", bufs=1))
        wo_sb = [wopool.tile([128, D_MODEL], BF16, name=f"wo{t}") for t in range(2)]
        for t in range(2):
            nc.sync.dma_start(wo_sb[t][:], wo_d.ap()[t * 128 : (t + 1) * 128, :])

        # ---- phase 2: attention per head pair ----
        _sid2, _ = nc.enter_named_scope("phase2", False)
        with (
            tc.tile_pool(name="spsum", bufs=2, space="PSUM") as spsum,
            tc.tile_pool(name="apsum", bufs=2, space="PSUM") as apsum,
            tc.tile_pool(name="epool", bufs=3) as epool,
            tc.tile_pool(name="rpool", bufs=2) as rpool,
        ):
            for pr in range(2):  # heads (2pr, 2pr+1), channels in qT[pr]/kT[pr]
                for i in range(NI):
                    aa = {}
                    for u in range(2):
                        aa[u] = apsum.tile([DK + 1, IB], F32, name=f"a{u}")
                    for j in range(NJ):
                        # both heads' scores in one 2-bank psum tile -> one exp
                        ss = spsum.tile([128, 2 * IB], F32, name="ss")
                        for u in range(2):
                            nc.tensor.matmul(
                                ss[:, u * IB : (u + 1) * IB],
                                kT[pr][u * DK : (u + 1) * DK, j * 128 : (j + 1) * 128],
                                qT[pr][u * DK : (u + 1) * DK, i * IB : (i + 1) * IB],
                                start=True,
                                stop=True,
                                tile_position=(u * DK, 0),
                            )
                        ee = epool.tile([128, 2 * IB], BF16, name="ee")
                        nc.scalar.activation(ee[:], ss[:], AF.Exp)
                        for u in range(2):
                            h = 2 * pr + u  # local head
                            nc.tensor.matmul(
                                aa[u][:],
                                v_sb[:, j * VW + h * (DK + 1) : j * VW + (h + 1) * (DK + 1)],
                                ee[:, u * IB : (u + 1) * IB],
                                start=(j == 0),
                                stop=(j == NJ - 1),
                            )
                    for u in range(2):
                        h = 2 * pr + u
                        # reciprocal of the sums row (PSUM partition 64), kept
                        # at base 64 so the DVE needs no partition shift; the
                        # row then moves to partition 0 by DMA because gpsimd
                        # partition_broadcast only accepts a base-0 source
                        r = rpool.tile([128, IB], F32, name="r")
                        nc.vector.reciprocal(r[DK : DK + 1, :], aa[u][DK : DK + 1, :])
                        r0 = rpool.tile([1, IB], F32, name="r0")
                        nc.sync.dma_start(r0[:], r[DK : DK + 1, :])
                        rb = rpool.tile([DK, IB], F32, name="rb")
                        nc.gpsimd.partition_broadcast(rb[:], r0[:])
                        t, po = h // 2, 64 * (h % 2)
                        if po == 0:
                            nc.vector.tensor_mul(
                                att[t][0:DK, i * IB : (i + 1) * IB], aa[u][0:DK, :], rb[:]
                            )
                        else:
                            nrm = rpool.tile([DK, IB], BF16, name="nrm")
                            nc.vector.tensor_mul(nrm[:], aa[u][0:DK, :], rb[:])
                            nc.sync.dma_start(
                                att[t][po : po + DK, i * IB : (i + 1) * IB], nrm[:]
                            )

        nc.leave_named_scope("phase2", _sid2, False)

        # ---- phase 3: out[i, m] = sum_c att[c, i] wo[c, m] ----
        _sid3, _ = nc.enter_named_scope("phase3", False)
        with (
            tc.tile_pool(name="opsum", bufs=2, space="PSUM") as opsum,
            tc.tile_pool(name="obounce", bufs=4) as obounce,
        ):
            for ib in range(S // 128):
                for mh in range(2):
                    po = opsum.tile([128, IB], F32, name=f"o{mh}")
                    for t in range(2):
                        nc.tensor.matmul(
                            po[:],
                            att[t][:, ib * 128 : (ib + 1) * 128],
                            wo_sb[t][:, mh * IB : (mh + 1) * IB],
                            start=(t == 0),
                            stop=(t == 1),
                        )
                    ob = obounce.tile([128, IB], F32, name="ob")
                    nc.vector.tensor_copy(ob[:], po[:])
                    nc.sync.dma_start(
                        out_d.ap()[ib * 128 : (ib + 1) * 128, mh * IB : (mh + 1) * IB],
                        ob[:],
                    )
        nc.leave_named_scope("phase3", _sid3, False)

    nc.compile()
    return nc


def _prep_inputs(Q, K, V, Wq, bq, Wk, bk, Wv, bv, Wo, bo):
    """Build the 8 per-core input maps (host-side shard + layout)."""
    bf16 = ml_dtypes.bfloat16
    ones = np.ones((1, S), dtype=np.float32)
    per_batch = []
    for b in range(B):
        xq = np.ascontiguousarray(Q[b].T).astype(bf16)
        xk = np.ascontiguousarray(K[b].T).astype(bf16)
        xv = np.concatenate([np.ascontiguousarray(V[b].T), ones], axis=0).astype(bf16)
        per_batch.append((xq, xk, xv))
    in_maps = []
    for c in range(8):
        b, g = divmod(c, NG)
        xq, xk, xv = per_batch[b]
        gs = slice(g * CG, (g + 1) * CG)
        wq = np.ascontiguousarray(Wq[:, gs]) * 0.125
        wk = np.ascontiguousarray(Wk[:, gs])
        bqk = np.stack([bq[gs] * 0.125, bk[gs]], axis=1).astype(np.float32)
        wv = np.zeros((KIN, VW), dtype=np.float32)
        for e in range(HPG):
            wv[:D_MODEL, e * (DK + 1) : e * (DK + 1) + DK] = Wv[
                :, g * CG + e * DK : g * CG + (e + 1) * DK
            ]
            wv[D_MODEL, e * (DK + 1) : e * (DK + 1) + DK] = bv[
                g * CG + e * DK : g * CG + (e + 1) * DK
            ]
            wv[D_MODEL, e * (DK + 1) + DK] = 1.0
        wo = np.ascontiguousarray(Wo[g * CG : (g + 1) * CG, :])
        in_maps.append(
            {
                "xq": xq,
                "xk": xk,
                "xv": np.ascontiguousarray(xv),
                "wq": wq.astype(bf16),
                "wk": wk.astype(bf16),
                "bqk": bqk,
                "wv": wv.astype(bf16),
                "wo": wo.astype(bf16),
            }
        )
    return in_maps


def run(inputs, trace=False):
    if "nc" not in _CACHE:
        _CACHE["nc"] = build_program()
    nc = _CACHE["nc"]
    in_maps = _prep_inputs(**inputs)
    res = run_bass_kernel_spmd(nc, in_maps, core_ids=list(range(8)), trace=trace)
    bo = np.asarray(inputs["bo"], dtype=np.float32)
    outs = []
    for b in range(B):
        acc = res.results[4 * b]["out"].astype(np.float32)
        for g in range(1, NG):
            acc = acc + res.results[4 * b + g]["out"]
        outs.append(acc + bo[None, :])
    return np.stack(outs, axis=0), res


def kernel(**inputs):
    inputs = {k: np.asarray(v) for k, v in inputs.items()}
    out, _ = run(inputs, trace=False)
    return out.astype(np.float32)
